# revision 1
# baseline (speedup 1.0000x reference)
"""Derivative1D kernel for Trainium2 (8 NeuronCores, data-parallel over batch).

Reference: y = x[:, 1:, :] - x[:, :-1, :] with x of shape (64, 16384, 32) f32.

Key observation: flattening each batch's (L, C) block to a contiguous array,
y_flat[i] = x_flat[i + C] - x_flat[i].  The row the reference drops (l = L-1)
absorbs the batch-boundary garbage, so the whole per-core problem is one flat
shifted subtraction; the garbage rows are sliced off on the host.

Sharding: batch axis across 8 cores (8 batches per core, no communication).

Per core the problem is pure DMA streaming: 16.8 MB in + 16.8 MB out through
16 SDMA engines that each sustain ~27.1 GB/s regardless of descriptor size,
plus a small (~10-15 ns) fixed cost per descriptor.  The kernel therefore:

- uses the LARGEST legal descriptors: per-partition contiguous runs just
  under the 64 KiB SDMA descriptor limit.  Chunks of (16344, 16344, 80)
  free-dim elements x 128 partitions cover the 4M-element shard with 65504 B
  load descriptors (the +32-element halo keeps the shift-by-C inside each
  partition) and 65376 B store descriptors -- half the descriptor count of a
  32 KiB-descriptor layout, worth ~0.5-1 us.
- subtracts IN PLACE (out aliases the unshifted operand; the stream reads
  A[j] and A[j+C] strictly before writing A[j], and every overwritten
  element was already consumed), so no second SBUF tile is needed and every
  chunk gets a dedicated buffer: no write-after-read coupling anywhere.
- hoists the three load dma_starts into the ENTRY block, right after the SP
  engine's preamble_end marker (the same insertion point bacc uses for
  collectives, guaranteed after the per-kernel sem clear).  The loads then
  issue BEFORE the constructor's all-engine barrier instead of after it,
  starting payload descriptors ~1.2 us earlier.  Safe because the entry
  barrier's SP InstDrain fences only descriptor generation (HWDGE completion
  is certified by the explicit SS wait), and the loads' semaphore increments
  land mid-execution, long after any preamble register writes.
- keeps a single HWDGE ring so reads and writes alternate at whole-DMA
  granularity (a dual-ring split interleaves R/W at packet granularity on
  every SDMA engine and was measured worse).
- per-chunk load semaphores (not one shared counter): a shared counter can
  reach the wait threshold via increments from a LATER DMA while a straggler
  engine still owes descriptors for an earlier one.
- no_gpsimd_drain skips the expensive GpSimd dge_drain at block exit; HWDGE
  completion is certified by the explicit SS wait.

Measured decomposition at 88.4 us: ~7.3 us fixed runtime preamble (loader
barriers + icache fetch, outside the kernel IR) before the first payload
descriptor, ~78.7 us of fully saturated descriptor streaming (zero
inter-descriptor gaps on all 16 engines), ~2.1 us of completion-semaphore
propagation + exit barrier + postamble accounting.  Occasional slow runs
(~100-124 us) are external HBM/SDMA contention from co-tenants (strict
round-robin descriptor dispatch, so a straggling engine drags every DMA)
and are not kernel-addressable.
"""

import numpy as np

B, L, C = 64, 16384, 32
NCORES = 8
BLOC = B // NCORES            # batches per core
N = BLOC * L * C              # flat elements per core
PAD = C                       # shift amount = channel count
P = 128                       # SBUF partitions
CHUNK_FS = [16344, 16344, 80]  # free-dim elements per chunk (desc < 64 KiB)
assert sum(CHUNK_FS) * P == N

_built = None


def build_bass():
    global _built
    if _built is not None:
        return _built
    import concourse.bass as bass
    import concourse.mybir as mybir
    from contextlib import ExitStack

    f32 = mybir.dt.float32
    nc = bass.Bass()
    x = nc.declare_dram_parameter("x", [N + PAD], f32, isOutput=False)
    y = nc.declare_dram_parameter("y", [N], f32, isOutput=True)

    nch = len(CHUNK_FS)
    offs = [P * sum(CHUNK_FS[:k]) for k in range(nch)]

    with ExitStack() as ctx:
        A = [
            ctx.enter_context(nc.sbuf_tensor(f"A{i}", [P, F + PAD], f32))
            for i, F in enumerate(CHUNK_FS)
        ]
        LS = [ctx.enter_context(nc.semaphore(f"LS{i}")) for i in range(nch)]
        SS = ctx.enter_context(nc.semaphore("SS"))
        VS = ctx.enter_context(nc.semaphore("VS"))

        block = ctx.enter_context(nc.Block(no_gpsimd_drain=True))

        @block.sync
        def _(sync):
            for k, F in enumerate(CHUNK_FS):
                sync.dma_start(
                    out=A[k][:],
                    in_=bass.AP(x, offs[k], [[F, P], [1, F + PAD]]),
                ).then_inc(LS[k], 16)
            for k, F in enumerate(CHUNK_FS):
                sync.wait_ge(VS, k + 1)
                sync.dma_start(
                    out=bass.AP(y, offs[k], [[F, P], [1, F]]),
                    in_=A[k][:, 0:F],
                ).then_inc(SS, 16)
            # All stores complete before the kernel exits.
            sync.wait_ge(SS, 16 * nch)

        @block.vector
        def _(vector):
            for k, F in enumerate(CHUNK_FS):
                vector.wait_ge(LS[k], 16)
                a = A[k]
                vector.tensor_sub(
                    a[:, 0:F], a[:, PAD : F + PAD], a[:, 0:F]
                ).then_inc(VS, 1)

    # Head surgery: move the three load DMAs from the SP body block into the
    # entry block, right after SP's preamble_end, so they issue before the
    # entry barrier (see module docstring for the safety argument).
    f = nc.main_func
    entry = f.blocks[0]
    spb = next(b for b in f.blocks if "_SP_" in b.name)
    loads = list(spb.instructions[:nch])
    assert all(type(i).__name__ == "InstDMACopy" for i in loads)
    for inst in loads:
        spb.instructions.remove(inst)
    idx = entry.instructions.index(nc.sync.preamble_end) + 1
    for j, inst in enumerate(loads):
        entry.instructions.insert(idx + j, inst)

    _built = nc
    return nc


def _shard_inputs(x: np.ndarray) -> list[dict]:
    in_maps = []
    for c in range(NCORES):
        shard = np.empty(N + PAD, dtype=np.float32)
        shard[:N] = x[c * BLOC : (c + 1) * BLOC].reshape(-1)
        shard[N:] = 0.0
        in_maps.append({"x": shard})
    return in_maps


def _gather_outputs(results: list[dict]) -> np.ndarray:
    y = np.empty((B, L - 1, C), dtype=np.float32)
    for c in range(NCORES):
        y[c * BLOC : (c + 1) * BLOC] = (
            results[c]["y"].reshape(BLOC, L, C)[:, : L - 1, :]
        )
    return y


def kernel(x: np.ndarray) -> np.ndarray:
    from concourse.bass_utils import run_bass_kernel_spmd

    nc = build_bass()
    x = np.asarray(x, dtype=np.float32)
    res = run_bass_kernel_spmd(nc, _shard_inputs(x), list(range(NCORES)))
    return _gather_outputs(res.results)



# revision 3
# speedup vs baseline: 1.0051x; 1.0051x over previous
"""Derivative1D kernel for Trainium2 (8 NeuronCores, data-parallel over batch).

Reference: y = x[:, 1:, :] - x[:, :-1, :] with x of shape (64, 16384, 32) f32.

Key observation: flattening each batch's (L, C) block to a contiguous array,
y_flat[i] = x_flat[i + C] - x_flat[i].  The row the reference drops (l = L-1)
absorbs the batch-boundary garbage, so the whole per-core problem is one flat
shifted subtraction; the garbage rows are sliced off on the host.

Sharding: batch axis across 8 cores (8 batches per core, no communication).

Per core the problem is pure DMA streaming: 16.8 MB in + 16.8 MB out through
16 SDMA engines that each sustain ~27.1 GB/s regardless of descriptor size,
plus a small (~10-15 ns) fixed cost per descriptor.  The kernel therefore:

- uses the LARGEST legal descriptors: per-partition contiguous runs just
  under the 64 KiB SDMA descriptor limit.  Chunks of (16344, 16344, 80)
  free-dim elements x 128 partitions cover the 4M-element shard with 65504 B
  load descriptors (the +32-element halo keeps the shift-by-C inside each
  partition) and 65376 B store descriptors -- half the descriptor count of a
  32 KiB-descriptor layout, worth ~0.5-1 us.
- subtracts IN PLACE (out aliases the unshifted operand; the stream reads
  A[j] and A[j+C] strictly before writing A[j], and every overwritten
  element was already consumed), so no second SBUF tile is needed and every
  chunk gets a dedicated buffer: no write-after-read coupling anywhere.
- hoists the three load dma_starts into the ENTRY block, right after the SP
  engine's preamble_end marker (the same insertion point bacc uses for
  collectives, guaranteed after the per-kernel sem clear).  The loads then
  issue BEFORE the constructor's all-engine barrier instead of after it,
  starting payload descriptors ~1.2 us earlier.  Safe because the entry
  barrier's SP InstDrain fences only descriptor generation (HWDGE completion
  is certified by the explicit SS wait), and the loads' semaphore increments
  land mid-execution, long after any preamble register writes.
- keeps a single HWDGE ring so reads and writes alternate at whole-DMA
  granularity (a dual-ring split interleaves R/W at packet granularity on
  every SDMA engine and was measured worse).
- per-chunk load semaphores (not one shared counter): a shared counter can
  reach the wait threshold via increments from a LATER DMA while a straggler
  engine still owes descriptors for an earlier one.
- no_gpsimd_drain skips the expensive GpSimd dge_drain at block exit; HWDGE
  completion is certified by the explicit SS wait.

Measured decomposition at 88.4 us: ~7.3 us fixed runtime preamble (loader
barriers + icache fetch, outside the kernel IR) before the first payload
descriptor, ~78.7 us of fully saturated descriptor streaming (zero
inter-descriptor gaps on all 16 engines), ~2.1 us of completion-semaphore
propagation + exit barrier + postamble accounting.  Occasional slow runs
(~100-124 us) are external HBM/SDMA contention from co-tenants (strict
round-robin descriptor dispatch, so a straggling engine drags every DMA)
and are not kernel-addressable.
"""

import numpy as np

B, L, C = 64, 16384, 32
NCORES = 8
BLOC = B // NCORES            # batches per core
N = BLOC * L * C              # flat elements per core
PAD = C                       # shift amount = channel count
P = 128                       # SBUF partitions
CHUNK_FS = [16344, 16344, 80]  # free-dim elements per chunk (desc < 64 KiB)
assert sum(CHUNK_FS) * P == N

_built = None


def build_bass():
    global _built
    if _built is not None:
        return _built
    import concourse.bass as bass
    import concourse.mybir as mybir
    from contextlib import ExitStack

    f32 = mybir.dt.float32
    nc = bass.Bass()
    x = nc.declare_dram_parameter("x", [N + PAD], f32, isOutput=False)
    y = nc.declare_dram_parameter("y", [N], f32, isOutput=True)

    nch = len(CHUNK_FS)
    offs = [P * sum(CHUNK_FS[:k]) for k in range(nch)]

    with ExitStack() as ctx:
        A = [
            ctx.enter_context(nc.sbuf_tensor(f"A{i}", [P, F + PAD], f32))
            for i, F in enumerate(CHUNK_FS)
        ]
        # Pin every kernel semaphore into [207, 255]: the walrus epilogue
        # clears the 256-sem file in fixed per-engine ranges (PE 2-53,
        # ACT 54-104, PL 105-155, DVE 156-206, SP 207-255), each engine
        # sweeping right after its own last program instruction.  With our
        # sems in SP's range, only SP -- which ends on the SS>=48 wait,
        # after all sem traffic has quiesced -- ever clears a live sem, so
        # the block-end all-engine barrier below can be stripped (see the
        # tail surgery after the Block).
        LS = [ctx.enter_context(nc.semaphore(f"LS{i}", num=207 + i)) for i in range(nch)]
        SS = ctx.enter_context(nc.semaphore("SS", num=210))
        VS = ctx.enter_context(nc.semaphore("VS", num=211))

        block = ctx.enter_context(nc.Block(no_gpsimd_drain=True))
        end_bb_name = block.end_bb

        @block.sync
        def _(sync):
            for k, F in enumerate(CHUNK_FS):
                sync.dma_start(
                    out=A[k][:],
                    in_=bass.AP(x, offs[k], [[F, P], [1, F + PAD]]),
                ).then_inc(LS[k], 16)
            for k, F in enumerate(CHUNK_FS):
                sync.wait_ge(VS, k + 1)
                sync.dma_start(
                    out=bass.AP(y, offs[k], [[F, P], [1, F]]),
                    in_=A[k][:, 0:F],
                ).then_inc(SS, 16)
            # All stores complete before the kernel exits.
            sync.wait_ge(SS, 16 * nch)

        @block.vector
        def _(vector):
            for k, F in enumerate(CHUNK_FS):
                vector.wait_ge(LS[k], 16)
                a = A[k]
                vector.tensor_sub(
                    a[:, 0:F], a[:, PAD : F + PAD], a[:, 0:F]
                ).then_inc(VS, 1)

    # Head surgery: move the three load DMAs from the SP body block into the
    # entry block, right after SP's preamble_end, so they issue before the
    # entry barrier (see module docstring for the safety argument).
    f = nc.main_func
    entry = f.blocks[0]
    spb = next(b for b in f.blocks if "_SP_" in b.name)
    loads = list(spb.instructions[:nch])
    assert all(type(i).__name__ == "InstDMACopy" for i in loads)
    for inst in loads:
        spb.instructions.remove(inst)
    idx = entry.instructions.index(nc.sync.preamble_end) + 1
    for j, inst in enumerate(loads):
        entry.instructions.insert(idx + j, inst)

    # Tail surgery: strip the block-end all-engine barrier (4 InstDrain +
    # the sem-only butterfly).  Without it each engine falls straight from
    # its last body instruction into the walrus epilogue (its ~50-sem clear
    # sweep + the S[2] value-sequenced exit barrier), so the sweeps of the
    # four early-finishing engines overlap the DMA streaming instead of
    # serializing behind SP's SS>=48 wait -- only SP's own 49-clear sweep
    # (~2.2 us) remains after store certification.  Safe because (a) our
    # sems live in SP's sweep range only (see above), (b) every other
    # engine's range holds sems that are zero/idle for the whole body, and
    # (c) the S[2] exit barrier is a value-sequenced handshake that
    # tolerates arbitrary per-engine arrival skew.
    endb = next(b for b in f.blocks if b.name == end_bb_name)
    endb.instructions[:] = []

    _built = nc
    return nc


def _shard_inputs(x: np.ndarray) -> list[dict]:
    in_maps = []
    for c in range(NCORES):
        shard = np.empty(N + PAD, dtype=np.float32)
        shard[:N] = x[c * BLOC : (c + 1) * BLOC].reshape(-1)
        shard[N:] = 0.0
        in_maps.append({"x": shard})
    return in_maps


def _gather_outputs(results: list[dict]) -> np.ndarray:
    y = np.empty((B, L - 1, C), dtype=np.float32)
    for c in range(NCORES):
        y[c * BLOC : (c + 1) * BLOC] = (
            results[c]["y"].reshape(BLOC, L, C)[:, : L - 1, :]
        )
    return y


def kernel(x: np.ndarray) -> np.ndarray:
    from concourse.bass_utils import run_bass_kernel_spmd

    nc = build_bass()
    x = np.asarray(x, dtype=np.float32)
    res = run_bass_kernel_spmd(nc, _shard_inputs(x), list(range(NCORES)))
    return _gather_outputs(res.results)

